# revision 61
# baseline (speedup 1.0000x reference)
"""GQA (32 Q heads / 8 KV heads, S=2048, D=4096, hd=128) on 8 TRN2 cores.

Tensor-parallel over heads: core c owns Q heads [4c, 4c+4) and KV head c.
Per-core bass kernel computes, fully on-chip (bf16 matmuls, fp32 accum):
  phase 1: qT/kT/vT projections (transposed layout [feat, seq]) + RoPE
  phase 2: causal attention, scoresT[j,i] layout, exp-softmax without
           max-subtraction (logits are small), PE ones-matmul row sums
  phase 3: partial output projection -> outT [4096, 2048] (fp32)
Host sums the 8 partial outputs and transposes back.

All layout transposes (x.T, W.T) are done on host; masks precomputed.
"""

import numpy as np

import concourse.bass as bass
import concourse.mybir as mybir
import concourse.tile as tile
from concourse import bacc
from concourse.bass_utils import run_bass_kernel_spmd
from concourse.masks import make_identity

B, S, D = 1, 2048, 4096
N_HEADS, N_KV = 32, 8
HD = 128                      # head dim
GROUP = N_HEADS // N_KV       # 4
NCORES = 8
HPC = N_HEADS // NCORES       # 4 q heads per core
QO = HPC * HD                 # 512 q rows per core
SCALE = 1.0 / np.sqrt(np.float32(HD))

SP = 256                      # phase-1 seq panel width
NSP = S // SP                 # 8 panels
DCH = D // 128                # 32 contraction chunks
IC = 512                      # phase-2 query chunk width
NIC = S // IC                 # 4 query chunks
NJB = S // 128                # 16 key blocks

BF = mybir.dt.bfloat16
F32 = mybir.dt.float32

DEBUG_TAPS = False  # extra dram outputs of qT/kT/vnat/ctx for CoreSim checks


def _build_nc():
    nc = bacc.Bacc("TRN2", target_bir_lowering=False, debug=False)

    xT = nc.dram_tensor("xT", [D, S], BF, kind="ExternalInput")
    cosT = nc.dram_tensor("cosT", [HD, S], BF, kind="ExternalInput")
    sinTs = nc.dram_tensor("sinTs", [HD, S], BF, kind="ExternalInput")
    # weights pre-arranged host-side into their SBUF layouts so each is one
    # big-element (full-bandwidth) DMA
    wqT = nc.dram_tensor("wqT", [HPC, 128, DCH, HD], BF, kind="ExternalInput")
    wkT = nc.dram_tensor("wkT", [128, DCH, HD], BF, kind="ExternalInput")
    wvT = nc.dram_tensor("wvT", [128, DCH, HD], BF, kind="ExternalInput")
    woT = nc.dram_tensor("woT", [128, HPC, D], BF, kind="ExternalInput")
    masks = nc.dram_tensor("masks", [128, 128], BF, kind="ExternalInput")
    outT = nc.dram_tensor("outT", [D, S], F32, kind="ExternalOutput")
    taps = None
    if DEBUG_TAPS:
        taps = {
            "dbg_q": nc.dram_tensor("dbg_q", [128, HPC, S], BF, kind="ExternalOutput"),
            "dbg_k": nc.dram_tensor("dbg_k", [128, S], BF, kind="ExternalOutput"),
            "dbg_v": nc.dram_tensor("dbg_v", [128, NJB, HD], BF, kind="ExternalOutput"),
            "dbg_ctx": nc.dram_tensor("dbg_ctx", [128, HPC, S], BF, kind="ExternalOutput"),
        }

    with tile.TileContext(nc) as tc:
        _emit(nc, tc, xT, cosT, sinTs, wqT, wkT, wvT, woT, masks, outT, taps)
    nc.compile()
    return nc


def _emit(nc, tc, xT, cosT, sinTs, wqT, wkT, wvT, woT, masks, outT, taps=None):
    from contextlib import ExitStack

    with ExitStack() as outer:
        ep = outer.enter_context  # persistent pools

        pers = ep(tc.tile_pool(name="pers", bufs=1))
        # per-chunk tiles so cross-phase deps resolve at chunk granularity
        # (the tile framework tracks whole tiles, not subranges)
        qTc = [pers.tile([128, HPC, IC], BF, name=f"qT{c}") for c in range(NIC)]
        kTc = [pers.tile([128, IC], BF, name=f"kT{c}") for c in range(NIC)]
        vnatc = [
            pers.tile([128, 4, HD], BF, name=f"vnat{c}") for c in range(NIC)
        ]
        ctxc = [
            pers.tile([128, HPC, IC], BF, name=f"ctx{c}") for c in range(NIC)
        ]
        mask_sb = pers.tile([128, 128], BF, name="mask_sb")
        ones_sb = pers.tile([128, 1], BF, name="ones_sb")
        ones1_sb = pers.tile([1, 128], BF, name="ones1_sb")
        ident = pers.tile([128, 128], BF, name="ident")
        woT_sb = pers.tile([128, HPC, D], BF, name="woT_sb")  # [m, head, o]
        warm_sb = pers.tile([1, 1], F32, name="warm_sb")

        nc.gpsimd.memset(ones_sb[:], 1.0)
        nc.gpsimd.memset(ones1_sb[:], 1.0)
        make_identity(nc, ident[:])

        # ---------------- phase 1: projections + RoPE ----------------
        with ExitStack() as p1:
            e = p1.enter_context
            # wq split per head, wk split in d-halves: finer tiles let the
            # first matmuls start as soon as their slice of weights lands
            # (tile deps are whole-tile)
            wq_pool = e(tc.tile_pool(name="wq", bufs=1))
            wq_sbs = [
                wq_pool.tile([128, DCH, HD], BF, name=f"wq_sb{h}")
                for h in range(HPC)
            ]
            wk_pool = e(tc.tile_pool(name="wk", bufs=1))
            wk_sbs = [
                wk_pool.tile([128, DCH // 2, HD], BF, name=f"wk_sb{u}")
                for u in range(2)
            ]
            wv_sb = e(tc.tile_pool(name="wv", bufs=1)).tile(
                [128, DCH, HD], BF, name="wv_sb"
            )
            cs_pool = e(tc.tile_pool(name="cs", bufs=1))
            cosT_sb = cs_pool.tile([128, S], BF, name="cosT_sb")
            sinTs_sb = cs_pool.tile([128, S], BF, name="sinTs_sb")



            xp_pool = e(tc.tile_pool(name="xp", bufs=2))
            p1_psum = e(tc.tile_pool(name="p1ps", bufs=6, space="PSUM"))
            tr_psum = e(tc.tile_pool(name="trps", bufs=2, space="PSUM"))
            rtmp_pool = e(tc.tile_pool(name="rtmp", bufs=2))
            vt_pool = e(tc.tile_pool(name="vt", bufs=2))

            def load_xpan(sp, halves=False):
                src = xT[:, sp * SP : (sp + 1) * SP].rearrange(
                    "(c p) s -> p c s", p=128
                )
                if not halves:
                    t = xp_pool.tile([128, DCH, SP], BF, name="xpan")
                    nc.sync.dma_start(t[:], src)
                    return [(t, 0)]
                # panel 0: two half tiles so the first K matmuls start after
                # only half the panel has landed
                out = []
                for u in range(2):
                    t = xp_pool.tile([128, DCH // 2, SP], BF, name="xpanh")
                    nc.sync.dma_start(
                        t[:], src[:, u * (DCH // 2) : (u + 1) * (DCH // 2), :]
                    )
                    out.append((t, u * (DCH // 2)))
                    if u == 0:
                        nc.sync.dma_start(wk_sbs[1][:], wkT[:, DCH // 2 :, :])
                return out

            def xsel(xpan, d):
                # xpan is a list of (tile, d_base); find the piece holding d
                for t, base in reversed(xpan):
                    if d >= base:
                        return t[:, d - base, :]
                raise AssertionError

            # DMA order = need order: the panel-0 K matmuls need only the
            # first wk/xpan0 halves; V/Q weights and rope tables stream in
            # behind them; wo (phase 3) last.
            nc.sync.dma_start(wk_sbs[0][:], wkT[:, : DCH // 2, :])
            xpan_next = load_xpan(0, halves=True)  # also queues wk_sbs[1]
            nc.sync.dma_start(wv_sb[:], wvT[:])
            nc.sync.dma_start(wq_sbs[0][:], wqT[0])
            nc.sync.dma_start(wq_sbs[1][:], wqT[1])
            nc.sync.dma_start(cosT_sb[:], cosT[:])
            nc.sync.dma_start(sinTs_sb[:], sinTs[:])
            nc.sync.dma_start(wq_sbs[2][:], wqT[2])
            nc.sync.dma_start(wq_sbs[3][:], wqT[3])
            xpan_1 = load_xpan(1)
            nc.sync.dma_start(mask_sb[:], masks[:])

            # Warm-up ops: absorb first-touch DMA waits on DVE/ACT so hot
            # TensorTensor/Activation instructions carry at most one wait
            # (the DVE TT struct rejects >1 sync wait in walrus codegen).
            # Self-writes create RAW deps that force these to run first.
            nc.vector.tensor_copy(cosT_sb[:1, :1], cosT_sb[:1, :1])
            nc.vector.tensor_copy(sinTs_sb[:1, :1], sinTs_sb[:1, :1])
            nc.vector.tensor_copy(mask_sb[:1, :1], mask_sb[:1, :1])
            nc.scalar.activation(
                warm_sb[:], cosT_sb[:1, :1], mybir.ActivationFunctionType.Copy
            )
            nc.scalar.activation(
                warm_sb[:], warm_sb[:], mybir.ActivationFunctionType.Exp
            )

            for sp in range(NSP):
                s0 = sp * SP
                xpan = xpan_next
                if sp == 0:
                    xpan_next = xpan_1  # already queued in the preamble
                elif sp + 1 < NSP:
                    xpan_next = load_xpan(sp + 1)
                if sp == 1:
                    # phase-3 weights, queued once the startup-critical
                    # DMAs are all in flight
                    nc.sync.dma_start(woT_sb[:], woT[:])
                ch, co = sp // 2, (sp % 2) * SP  # chunk tile + col offset

                def k_block(xpan=xpan, s0=s0, ch=ch, co=co):
                    ps = p1_psum.tile([128, SP], F32, name="p1acc", tag="p1acc")
                    for d in range(DCH):
                        nc.tensor.matmul(
                            ps[:], wk_sbs[d // (DCH // 2)][:, d % (DCH // 2), :],
                            xsel(xpan, d),
                            start=(d == 0), stop=(d == DCH - 1),
                        )
                    _rope(nc, tc, rtmp_pool, ps, kTc[ch][:, co : co + SP],
                          cosT_sb[:, s0 : s0 + SP], sinTs_sb[:, s0 : s0 + SP])

                def v_block(xpan=xpan, s0=s0):
                    # evac then PE-transpose to natural [s, hd]
                    ps = p1_psum.tile([128, SP], F32, name="p1acc", tag="p1acc")
                    for d in range(DCH):
                        nc.tensor.matmul(
                            ps[:], wv_sb[:, d, :], xsel(xpan, d),
                            start=(d == 0), stop=(d == DCH - 1),
                        )
                    vt = vt_pool.tile([128, SP], BF, name="vt")
                    nc.scalar.activation(
                        vt[:], ps[:], mybir.ActivationFunctionType.Copy
                    )
                    for b in range(SP // 128):
                        jb = (s0 + b * 128) // 128
                        tp = tr_psum.tile([128, 128], BF, name="trp", tag="trp")
                        nc.tensor.transpose(
                            tp[:], vt[:, b * 128 : (b + 1) * 128], ident[:]
                        )
                        nc.scalar.activation(
                            vnatc[jb // 4][:, jb % 4, :],
                            tp[:],
                            mybir.ActivationFunctionType.Copy,
                        )

                def q_block(h, xpan=xpan, s0=s0, ch=ch, co=co):
                    ps = p1_psum.tile([128, SP], F32, name="p1acc", tag="p1acc")
                    for d in range(DCH):
                        nc.tensor.matmul(
                            ps[:],
                            wq_sbs[h][:, d, :],
                            xsel(xpan, d),
                            start=(d == 0),
                            stop=(d == DCH - 1),
                        )
                    _rope(nc, tc, rtmp_pool, ps, qTc[ch][:, h, co : co + SP],
                          cosT_sb[:, s0 : s0 + SP], sinTs_sb[:, s0 : s0 + SP])

                if sp < NSP - 1:
                    k_block()
                    v_block()
                    for h in range(HPC):
                        q_block(h)
                else:
                    # last panel: Q (long DVE rope chains) first so the final
                    # p1-psum consumers finish close to the last PE matmul —
                    # phase 2's psum pool waits on this pool's release
                    for h in range(HPC):
                        q_block(h)
                    k_block()
                    v_block()

        # ---------------- phase 2: attention ----------------
        # Off-diagonal key blocks are processed in PAIRS sharing one batched
        # exp (halves ACT instruction overhead). Diagonal blocks stream only
        # the causally-valid query columns [off*128:IC] through scores / exp /
        # ctx / rowsum; the remaining triangle is zeroed by a [128,128] mask.
        with ExitStack() as p2:
            e = p2.enter_context
            sc_psum = e(tc.tile_pool(name="scps", bufs=2, space="PSUM"))
            cx_psum = e(tc.tile_pool(name="cxps", bufs=2, space="PSUM"))
            rs_psum = e(tc.tile_pool(name="rsps", bufs=2, space="PSUM"))
            pt_pool = e(tc.tile_pool(name="pt", bufs=6))
            rs_pool = e(tc.tile_pool(name="rs", bufs=2))
            bc_pool = e(tc.tile_pool(name="bc", bufs=2))
            EXP = mybir.ActivationFunctionType.Exp

            def chunk_units(c, h):
                    q_rhs = qTc[c][:, h, :]
                    cps = cx_psum.tile([128, IC], F32, name="cxa", tag="cxa")
                    rps = rs_psum.tile([1, IC], F32, name="rsa", tag="rsa")

                    def scores_pair(g):
                        sps = sc_psum.tile(
                            [128, 2, IC], F32, name="scp", tag="scp"
                        )
                        for u in range(2):
                            jb = 2 * g + u
                            nc.tensor.matmul(
                                sps[:, u, :],
                                kTc[jb // 4][:, (jb % 4) * 128 : (jb % 4 + 1) * 128],
                                q_rhs,
                                start=True,
                                stop=True,
                            )
                        pt = pt_pool.tile([128, 2, IC], BF, name="pt", tag="pt")
                        nc.scalar.activation(pt[:], sps[:], EXP, scale=float(SCALE))
                        return pt

                    def tail_pair(g, pt, is_last=False):
                        for u in range(2):
                            jb = 2 * g + u
                            stop = is_last and u == 1
                            nc.tensor.matmul(
                                cps[:], vnatc[jb // 4][:, jb % 4, :], pt[:, u, :],
                                start=(jb == 0), stop=stop,
                                skip_group_check=True,
                            )
                            nc.tensor.matmul(
                                rps[:], ones_sb[:], pt[:, u, :],
                                start=(jb == 0), stop=stop,
                                skip_group_check=True,
                            )

                    def scores_diag(off):
                        lo = off * 128
                        sps = sc_psum.tile(
                            [128, 2, IC], F32, name="scp", tag="scp"
                        )
                        nc.tensor.matmul(
                            sps[:, 0, lo:IC],
                            kTc[c][:, lo : lo + 128],
                            q_rhs[:, lo:IC],
                            start=True,
                            stop=True,
                        )
                        pt = pt_pool.tile([128, 2, IC], BF, name="pt", tag="pt")
                        nc.scalar.activation(
                            pt[:, 0, lo:IC], sps[:, 0, lo:IC], EXP,
                            scale=float(SCALE),
                        )
                        nc.vector.tensor_mul(
                            pt[:, 0, lo : lo + 128],
                            pt[:, 0, lo : lo + 128],
                            mask_sb[:],
                        )
                        return pt

                    def tail_diag(off, pt, is_last=False):
                        lo = off * 128
                        first = c == 0 and off == 0
                        hi = lo + 128
                        if first:
                            # single full-width start=True write: a second
                            # start=True into the same psum bank would zero
                            # the previously written region
                            nc.tensor.matmul(
                                cps[:, lo:IC], vnatc[c][:, off, :],
                                pt[:, 0, lo:IC],
                                start=True, stop=is_last,
                                skip_group_check=True,
                            )
                            nc.tensor.matmul(
                                rps[:, lo:IC], ones_sb[:], pt[:, 0, lo:IC],
                                start=True, stop=is_last,
                                skip_group_check=True,
                            )
                            return
                        if hi < IC:
                            # mask-free columns: depend on exp only, keeping
                            # PE fed while the triangle mask runs on DVE
                            nc.tensor.matmul(
                                cps[:, hi:IC], vnatc[c][:, off, :],
                                pt[:, 0, hi:IC],
                                start=first, stop=False,
                                skip_group_check=True,
                            )
                            nc.tensor.matmul(
                                rps[:, hi:IC], ones_sb[:], pt[:, 0, hi:IC],
                                start=first, stop=False,
                                skip_group_check=True,
                            )
                        nc.tensor.matmul(
                            cps[:, lo:hi], vnatc[c][:, off, :], pt[:, 0, lo:hi],
                            start=first, stop=is_last, skip_group_check=True,
                        )
                        nc.tensor.matmul(
                            rps[:, lo:hi], ones_sb[:], pt[:, 0, lo:hi],
                            start=first, stop=is_last, skip_group_check=True,
                        )

                    def finalize():
                        rsf = rs_pool.tile([1, IC], F32, name="rsf", tag="rsf")
                        nc.vector.reciprocal(rsf[:], rps[:])
                        bc = bc_pool.tile([128, IC], F32, name="bc", tag="bc")
                        nc.gpsimd.partition_broadcast(bc[:], rsf[:])
                        nc.vector.tensor_mul(ctxc[c][:, h, :], cps[:], bc[:])

                    units = [
                        (scores_pair, tail_pair, g) for g in range(2 * c)
                    ] + [(scores_diag, tail_diag, off) for off in range(4)]
                    return units, finalize

            # Global software pipeline, lookahead 2 across ALL chunks: unit
            # i+2's score matmuls are emitted before unit i's ctx/rowsum so
            # the PE has ~2 units of exp-independent work while an exp is in
            # flight — including at chunk boundaries. (Score-psum buffer
            # reuse only needs exp(i) done before unit i+2's scores run,
            # which the same spacing guarantees.)
            from collections import deque

            pend = deque()
            for c in range(NIC):
                for h in range(HPC):
                    units, fin = chunk_units(c, h)
                    for i, (sc_fn, tl_fn, arg) in enumerate(units):
                        pt = sc_fn(arg)
                        if len(pend) >= 2:
                            p = pend.popleft()
                            p[1](p[2], p[0], is_last=p[3] is not None)
                            if p[3] is not None:
                                p[3]()
                        pend.append((pt, tl_fn, arg,
                                     fin if i == len(units) - 1 else None))
            while pend:
                p = pend.popleft()
                p[1](p[2], p[0], is_last=p[3] is not None)
                if p[3] is not None:
                    p[3]()

        if taps is not None:
            for c in range(NIC):
                nc.sync.dma_start(
                    taps["dbg_q"][:, :, c * IC : (c + 1) * IC], qTc[c][:]
                )
                nc.sync.dma_start(
                    taps["dbg_k"][:, c * IC : (c + 1) * IC], kTc[c][:]
                )
                nc.sync.dma_start(
                    taps["dbg_v"][:, c * 4 : (c + 1) * 4, :], vnatc[c][:]
                )
                nc.sync.dma_start(
                    taps["dbg_ctx"][:, :, c * IC : (c + 1) * IC], ctxc[c][:]
                )

        # ---------------- phase 3: output projection ----------------
        with ExitStack() as p3:
            e = p3.enter_context
            p3_psum = e(tc.tile_pool(name="p3ps", bufs=4, space="PSUM"))
            ev_pool = e(tc.tile_pool(name="ev", bufs=8))
            for c in range(NIC):
                s0 = c * IC
                for ob in range(D // 128):
                    ps = p3_psum.tile([128, IC], F32, name="p3a", tag="p3a")
                    for h in range(HPC):
                        nc.tensor.matmul(
                            ps[:],
                            woT_sb[:, h, ob * 128 : (ob + 1) * 128],
                            ctxc[c][:, h, :],
                            start=(h == 0),
                            stop=(h == HPC - 1),
                        )
                    ev = ev_pool.tile([128, IC], F32, name="ev", tag="ev")
                    nc.scalar.activation(
                        ev[:], ps[:], mybir.ActivationFunctionType.Copy
                    )
                    nc.sync.dma_start(
                        outT[ob * 128 : (ob + 1) * 128, s0 : s0 + IC], ev[:]
                    )


def _rope(nc, tc, rtmp_pool, ps, out_ap, cos_ap, sins_ap):
    """out = psum*cos + rot(psum)*sin_signed, written as bf16.

    sins_ap holds [-sin_lo; sin_hi]; rot(q) = [q_hi; q_lo] (64-row swap).
    """
    n = ps.shape[-1]
    tmp = rtmp_pool.tile([128, n], F32, name="ropetmp", tag="ropetmp")
    t2 = rtmp_pool.tile([128, n], F32, name="ropet2", tag="ropet2")
    nc.vector.tensor_mul(tmp[:], ps[:], cos_ap)
    nc.vector.tensor_mul(t2[:64, :], ps[64:, :], sins_ap[:64, :])
    nc.vector.tensor_mul(t2[64:, :], ps[:64, :], sins_ap[64:, :])
    nc.vector.tensor_add(out_ap, tmp[:], t2[:])


_NC_CACHE = None


def _get_nc():
    global _NC_CACHE
    if _NC_CACHE is None:
        _NC_CACHE = _build_nc()
    return _NC_CACHE


def prepare_in_maps(x, cos, sin, Wq, Wk, Wv, Wo):
    x = np.asarray(x, np.float32)
    cos = np.asarray(cos, np.float32)
    sin = np.asarray(sin, np.float32)
    Wq = np.asarray(Wq, np.float32)
    Wk = np.asarray(Wk, np.float32)
    Wv = np.asarray(Wv, np.float32)
    Wo = np.asarray(Wo, np.float32)

    import ml_dtypes

    bf = ml_dtypes.bfloat16
    xT_bf = np.ascontiguousarray(x[0].T).astype(bf)
    cosT = np.ascontiguousarray(cos.T).astype(bf)            # [128, S] bf16
    sinT = sin.T
    sinTs = np.ascontiguousarray(
        np.concatenate([-sinT[:64], sinT[64:]], axis=0)
    ).astype(bf)

    # causal triangle for a single [key 128, query 128] diagonal sub-block
    j = np.arange(128)[:, None]
    i = np.arange(128)[None, :]
    masks = (i >= j).astype(np.float32).astype(bf)

    def sbuf_layout(wT, width):
        # [D, width] -> [128, DCH, width] (partition-major SBUF image)
        return np.ascontiguousarray(
            wT.reshape(DCH, 128, width).transpose(1, 0, 2)
        ).astype(bf)

    in_maps = []
    for c in range(NCORES):
        wq_c = Wq[c * QO : (c + 1) * QO]          # [512, 4096]
        wk_c = Wk[c * HD : (c + 1) * HD]          # [128, 4096]
        wv_c = Wv[c * HD : (c + 1) * HD]
        wo_c = Wo[:, c * QO : (c + 1) * QO]       # [4096, 512]
        woT_pre = np.ascontiguousarray(
            wo_c.T.reshape(HPC, 128, D).transpose(1, 0, 2)
        ).astype(bf)                               # [128, HPC, D]
        in_maps.append(
            {
                "xT": xT_bf,
                "cosT": cosT,
                "sinTs": sinTs,
                "wqT": np.ascontiguousarray(
                    sbuf_layout(wq_c.T, QO)
                    .reshape(128, DCH, HPC, HD)
                    .transpose(2, 0, 1, 3)
                ),
                "wkT": sbuf_layout(wk_c.T, HD),
                "wvT": sbuf_layout(wv_c.T, HD),
                "woT": woT_pre,
                "masks": masks,
            }
        )
    return in_maps


def kernel(x, cos, sin, Wq, Wk, Wv, Wo, _trace=False):
    nc = _get_nc()
    in_maps = prepare_in_maps(x, cos, sin, Wq, Wk, Wv, Wo)
    res = run_bass_kernel_spmd(
        nc, in_maps, core_ids=list(range(NCORES)), trace=_trace
    )
    acc = np.zeros((D, S), np.float32)
    for r in res.results:
        acc += r["outT"]
    out = np.ascontiguousarray(acc.T)[None]      # [1, S, D] fp32
    if _trace:
        return out, res
    return out



# revision 65
# speedup vs baseline: 1.9274x; 1.9274x over previous
"""GQA (32 Q heads / 8 KV heads, S=2048, D=4096, hd=128) on 8 TRN2 cores.

Tensor-parallel over heads: core c owns Q heads [4c, 4c+4) and KV head c.
Per-core bass kernel computes, fully on-chip (bf16 matmuls, fp32 accum):
  phase 1: qT/kT/vT projections (transposed layout [feat, seq]) + RoPE
  phase 2: causal attention, scoresT[j,i] layout, exp-softmax without
           max-subtraction (logits are small), PE ones-matmul row sums
  phase 3: partial output projection -> outT [4096, 2048] (fp32)
Host sums the 8 partial outputs and transposes back.

All layout transposes (x.T, W.T) are done on host; masks precomputed.
"""

import numpy as np

import concourse.bass as bass
import concourse.mybir as mybir
import concourse.tile as tile
from concourse import bacc
from concourse.bass_utils import run_bass_kernel_spmd
from concourse.masks import make_identity

B, S, D = 1, 2048, 4096
N_HEADS, N_KV = 32, 8
HD = 128                      # head dim
GROUP = N_HEADS // N_KV       # 4
NCORES = 8
HPC = N_HEADS // NCORES       # 4 q heads per core
QO = HPC * HD                 # 512 q rows per core
SCALE = 1.0 / np.sqrt(np.float32(HD))

SP = 256                      # phase-1 seq panel width
NSP = S // SP                 # 8 panels
DCH = D // 128                # 32 contraction chunks
IC = 512                      # phase-2 query chunk width
NIC = S // IC                 # 4 query chunks
NJB = S // 128                # 16 key blocks

BF = mybir.dt.bfloat16
F32 = mybir.dt.float32

DEBUG_TAPS = False  # extra dram outputs of qT/kT/vnat/ctx for CoreSim checks


def _build_nc():
    nc = bacc.Bacc("TRN2", target_bir_lowering=False, debug=False)

    xT = nc.dram_tensor("xT", [D, S], BF, kind="ExternalInput")
    cosT = nc.dram_tensor("cosT", [HD, S], BF, kind="ExternalInput")
    sinTs = nc.dram_tensor("sinTs", [HD, S], BF, kind="ExternalInput")
    # weights pre-arranged host-side into their SBUF layouts so each is one
    # big-element (full-bandwidth) DMA
    wqT = nc.dram_tensor("wqT", [HPC, 128, DCH, HD], BF, kind="ExternalInput")
    wkT = nc.dram_tensor("wkT", [128, DCH, HD], BF, kind="ExternalInput")
    wvT = nc.dram_tensor("wvT", [128, DCH, HD], BF, kind="ExternalInput")
    woT = nc.dram_tensor("woT", [128, HPC, D], BF, kind="ExternalInput")
    masks = nc.dram_tensor("masks", [128, 128], BF, kind="ExternalInput")
    outT = nc.dram_tensor("outT", [D, S], BF, kind="ExternalOutput")
    taps = None
    if DEBUG_TAPS:
        taps = {
            "dbg_q": nc.dram_tensor("dbg_q", [128, HPC, S], BF, kind="ExternalOutput"),
            "dbg_k": nc.dram_tensor("dbg_k", [128, S], BF, kind="ExternalOutput"),
            "dbg_v": nc.dram_tensor("dbg_v", [128, NJB, HD], BF, kind="ExternalOutput"),
            "dbg_ctx": nc.dram_tensor("dbg_ctx", [128, HPC, S], BF, kind="ExternalOutput"),
        }

    with tile.TileContext(nc) as tc:
        _emit(nc, tc, xT, cosT, sinTs, wqT, wkT, wvT, woT, masks, outT, taps)
    nc.compile()
    return nc


def _emit(nc, tc, xT, cosT, sinTs, wqT, wkT, wvT, woT, masks, outT, taps=None):
    from contextlib import ExitStack

    with ExitStack() as outer:
        ep = outer.enter_context  # persistent pools

        pers = ep(tc.tile_pool(name="pers", bufs=1))
        # per-chunk tiles so cross-phase deps resolve at chunk granularity
        # (the tile framework tracks whole tiles, not subranges)
        qTc = [pers.tile([128, HPC, IC], BF, name=f"qT{c}") for c in range(NIC)]
        kTc = [pers.tile([128, IC], BF, name=f"kT{c}") for c in range(NIC)]
        vnatc = [
            pers.tile([128, 4, HD], BF, name=f"vnat{c}") for c in range(NIC)
        ]
        ctxc = [
            pers.tile([128, HPC, IC], BF, name=f"ctx{c}") for c in range(NIC)
        ]
        mask_sb = pers.tile([128, 128], BF, name="mask_sb")
        ones_sb = pers.tile([128, 1], BF, name="ones_sb")
        ones1_sb = pers.tile([1, 128], BF, name="ones1_sb")
        ident = pers.tile([128, 128], BF, name="ident")
        woT_sb = pers.tile([128, HPC, D], BF, name="woT_sb")  # [m, head, o]
        warm_sb = pers.tile([1, 1], F32, name="warm_sb")

        nc.gpsimd.memset(ones_sb[:], 1.0)
        nc.gpsimd.memset(ones1_sb[:], 1.0)
        make_identity(nc, ident[:])

        # ---------------- phase 1: projections + RoPE ----------------
        with ExitStack() as p1:
            e = p1.enter_context
            # wq split per head, wk split in d-halves: finer tiles let the
            # first matmuls start as soon as their slice of weights lands
            # (tile deps are whole-tile)
            wq_pool = e(tc.tile_pool(name="wq", bufs=1))
            wq_sbs = [
                wq_pool.tile([128, DCH, HD], BF, name=f"wq_sb{h}")
                for h in range(HPC)
            ]
            wk_pool = e(tc.tile_pool(name="wk", bufs=1))
            wk_sbs = [
                wk_pool.tile([128, DCH // 2, HD], BF, name=f"wk_sb{u}")
                for u in range(2)
            ]
            wv_sb = e(tc.tile_pool(name="wv", bufs=1)).tile(
                [128, DCH, HD], BF, name="wv_sb"
            )
            cs_pool = e(tc.tile_pool(name="cs", bufs=1))
            cosT_sb = cs_pool.tile([128, S], BF, name="cosT_sb")
            sinTs_sb = cs_pool.tile([128, S], BF, name="sinTs_sb")



            xp_pool = e(tc.tile_pool(name="xp", bufs=2))
            p1_psum = e(tc.tile_pool(name="p1ps", bufs=6, space="PSUM"))
            tr_psum = e(tc.tile_pool(name="trps", bufs=2, space="PSUM"))
            rtmp_pool = e(tc.tile_pool(name="rtmp", bufs=2))
            vt_pool = e(tc.tile_pool(name="vt", bufs=2))

            def load_xpan(sp, halves=False):
                src = xT[:, sp * SP : (sp + 1) * SP].rearrange(
                    "(c p) s -> p c s", p=128
                )
                if not halves:
                    t = xp_pool.tile([128, DCH, SP], BF, name="xpan")
                    nc.sync.dma_start(t[:], src)
                    return [(t, 0)]
                # panel 0: two half tiles so the first K matmuls start after
                # only half the panel has landed
                out = []
                for u in range(2):
                    t = xp_pool.tile([128, DCH // 2, SP], BF, name="xpanh")
                    nc.sync.dma_start(
                        t[:], src[:, u * (DCH // 2) : (u + 1) * (DCH // 2), :]
                    )
                    out.append((t, u * (DCH // 2)))
                    if u == 0:
                        nc.sync.dma_start(wk_sbs[1][:], wkT[:, DCH // 2 :, :])
                return out

            def xsel(xpan, d):
                # xpan is a list of (tile, d_base); find the piece holding d
                for t, base in reversed(xpan):
                    if d >= base:
                        return t[:, d - base, :]
                raise AssertionError

            # DMA order = need order: the panel-0 K matmuls need only the
            # first wk/xpan0 halves; V/Q weights and rope tables stream in
            # behind them; wo (phase 3) last.
            nc.sync.dma_start(wk_sbs[0][:], wkT[:, : DCH // 2, :])
            xpan_next = load_xpan(0, halves=True)  # also queues wk_sbs[1]
            nc.sync.dma_start(wv_sb[:], wvT[:])
            nc.sync.dma_start(wq_sbs[0][:], wqT[0])
            nc.sync.dma_start(wq_sbs[1][:], wqT[1])
            nc.sync.dma_start(cosT_sb[:], cosT[:])
            nc.sync.dma_start(sinTs_sb[:], sinTs[:])
            nc.sync.dma_start(wq_sbs[2][:], wqT[2])
            nc.sync.dma_start(wq_sbs[3][:], wqT[3])
            xpan_1 = load_xpan(1)
            nc.sync.dma_start(mask_sb[:], masks[:])

            # Warm-up ops: absorb first-touch DMA waits on DVE/ACT so hot
            # TensorTensor/Activation instructions carry at most one wait
            # (the DVE TT struct rejects >1 sync wait in walrus codegen).
            # Self-writes create RAW deps that force these to run first.
            nc.vector.tensor_copy(cosT_sb[:1, :1], cosT_sb[:1, :1])
            nc.vector.tensor_copy(sinTs_sb[:1, :1], sinTs_sb[:1, :1])
            nc.vector.tensor_copy(mask_sb[:1, :1], mask_sb[:1, :1])
            nc.scalar.activation(
                warm_sb[:], cosT_sb[:1, :1], mybir.ActivationFunctionType.Copy
            )
            nc.scalar.activation(
                warm_sb[:], warm_sb[:], mybir.ActivationFunctionType.Exp
            )

            for sp in range(NSP):
                s0 = sp * SP
                xpan = xpan_next
                if sp == 0:
                    xpan_next = xpan_1  # already queued in the preamble
                elif sp + 1 < NSP:
                    xpan_next = load_xpan(sp + 1)
                if sp == 1:
                    # phase-3 weights, queued once the startup-critical
                    # DMAs are all in flight
                    nc.sync.dma_start(woT_sb[:], woT[:])
                ch, co = sp // 2, (sp % 2) * SP  # chunk tile + col offset

                def k_block(xpan=xpan, s0=s0, ch=ch, co=co):
                    ps = p1_psum.tile([128, SP], F32, name="p1acc", tag="p1acc")
                    for d in range(DCH):
                        nc.tensor.matmul(
                            ps[:], wk_sbs[d // (DCH // 2)][:, d % (DCH // 2), :],
                            xsel(xpan, d),
                            start=(d == 0), stop=(d == DCH - 1),
                        )
                    _rope(nc, tc, rtmp_pool, ps, kTc[ch][:, co : co + SP],
                          cosT_sb[:, s0 : s0 + SP], sinTs_sb[:, s0 : s0 + SP])

                def v_block(xpan=xpan, s0=s0):
                    # evac then PE-transpose to natural [s, hd]
                    ps = p1_psum.tile([128, SP], F32, name="p1acc", tag="p1acc")
                    for d in range(DCH):
                        nc.tensor.matmul(
                            ps[:], wv_sb[:, d, :], xsel(xpan, d),
                            start=(d == 0), stop=(d == DCH - 1),
                        )
                    vt = vt_pool.tile([128, SP], BF, name="vt")
                    nc.scalar.activation(
                        vt[:], ps[:], mybir.ActivationFunctionType.Copy
                    )
                    for b in range(SP // 128):
                        jb = (s0 + b * 128) // 128
                        tp = tr_psum.tile([128, 128], BF, name="trp", tag="trp")
                        nc.tensor.transpose(
                            tp[:], vt[:, b * 128 : (b + 1) * 128], ident[:]
                        )
                        nc.scalar.activation(
                            vnatc[jb // 4][:, jb % 4, :],
                            tp[:],
                            mybir.ActivationFunctionType.Copy,
                        )

                def q_block(h, xpan=xpan, s0=s0, ch=ch, co=co):
                    ps = p1_psum.tile([128, SP], F32, name="p1acc", tag="p1acc")
                    for d in range(DCH):
                        nc.tensor.matmul(
                            ps[:],
                            wq_sbs[h][:, d, :],
                            xsel(xpan, d),
                            start=(d == 0),
                            stop=(d == DCH - 1),
                        )
                    _rope(nc, tc, rtmp_pool, ps, qTc[ch][:, h, co : co + SP],
                          cosT_sb[:, s0 : s0 + SP], sinTs_sb[:, s0 : s0 + SP])

                if sp < NSP - 1:
                    k_block()
                    v_block()
                    for h in range(HPC):
                        q_block(h)
                else:
                    # last panel: Q (long DVE rope chains) first so the final
                    # p1-psum consumers finish close to the last PE matmul —
                    # phase 2's psum pool waits on this pool's release
                    for h in range(HPC):
                        q_block(h)
                    k_block()
                    v_block()

        # ---------------- phase 2: attention ----------------
        # Off-diagonal key blocks are processed in PAIRS sharing one batched
        # exp (halves ACT instruction overhead). Diagonal blocks stream only
        # the causally-valid query columns [off*128:IC] through scores / exp /
        # ctx / rowsum; the remaining triangle is zeroed by a [128,128] mask.
        with ExitStack() as p2:
            e = p2.enter_context
            sc_psum = e(tc.tile_pool(name="scps", bufs=2, space="PSUM"))
            cx_psum = e(tc.tile_pool(name="cxps", bufs=2, space="PSUM"))
            rs_psum = e(tc.tile_pool(name="rsps", bufs=2, space="PSUM"))
            pt_pool = e(tc.tile_pool(name="pt", bufs=6))
            rs_pool = e(tc.tile_pool(name="rs", bufs=2))
            bc_pool = e(tc.tile_pool(name="bc", bufs=2))
            EXP = mybir.ActivationFunctionType.Exp

            def chunk_units(c, h):
                    q_rhs = qTc[c][:, h, :]
                    cps = cx_psum.tile([128, IC], F32, name="cxa", tag="cxa")
                    rps = rs_psum.tile([1, IC], F32, name="rsa", tag="rsa")

                    def scores_pair(g):
                        sps = sc_psum.tile(
                            [128, 2, IC], F32, name="scp", tag="scp"
                        )
                        for u in range(2):
                            jb = 2 * g + u
                            nc.tensor.matmul(
                                sps[:, u, :],
                                kTc[jb // 4][:, (jb % 4) * 128 : (jb % 4 + 1) * 128],
                                q_rhs,
                                start=True,
                                stop=True,
                            )
                        pt = pt_pool.tile([128, 2, IC], BF, name="pt", tag="pt")
                        nc.scalar.activation(pt[:], sps[:], EXP, scale=float(SCALE))
                        return pt

                    def tail_pair(g, pt, is_last=False):
                        for u in range(2):
                            jb = 2 * g + u
                            stop = is_last and u == 1
                            nc.tensor.matmul(
                                cps[:], vnatc[jb // 4][:, jb % 4, :], pt[:, u, :],
                                start=(jb == 0), stop=stop,
                                skip_group_check=True,
                            )
                            nc.tensor.matmul(
                                rps[:], ones_sb[:], pt[:, u, :],
                                start=(jb == 0), stop=stop,
                                skip_group_check=True,
                            )

                    def scores_diag(off):
                        lo = off * 128
                        sps = sc_psum.tile(
                            [128, 2, IC], F32, name="scp", tag="scp"
                        )
                        nc.tensor.matmul(
                            sps[:, 0, lo:IC],
                            kTc[c][:, lo : lo + 128],
                            q_rhs[:, lo:IC],
                            start=True,
                            stop=True,
                        )
                        pt = pt_pool.tile([128, 2, IC], BF, name="pt", tag="pt")
                        nc.scalar.activation(
                            pt[:, 0, lo:IC], sps[:, 0, lo:IC], EXP,
                            scale=float(SCALE),
                        )
                        nc.vector.tensor_mul(
                            pt[:, 0, lo : lo + 128],
                            pt[:, 0, lo : lo + 128],
                            mask_sb[:],
                        )
                        return pt

                    def tail_diag(off, pt, is_last=False):
                        lo = off * 128
                        first = c == 0 and off == 0
                        hi = lo + 128
                        if first:
                            # single full-width start=True write: a second
                            # start=True into the same psum bank would zero
                            # the previously written region
                            nc.tensor.matmul(
                                cps[:, lo:IC], vnatc[c][:, off, :],
                                pt[:, 0, lo:IC],
                                start=True, stop=is_last,
                                skip_group_check=True,
                            )
                            nc.tensor.matmul(
                                rps[:, lo:IC], ones_sb[:], pt[:, 0, lo:IC],
                                start=True, stop=is_last,
                                skip_group_check=True,
                            )
                            return
                        if hi < IC:
                            # mask-free columns: depend on exp only, keeping
                            # PE fed while the triangle mask runs on DVE
                            nc.tensor.matmul(
                                cps[:, hi:IC], vnatc[c][:, off, :],
                                pt[:, 0, hi:IC],
                                start=first, stop=False,
                                skip_group_check=True,
                            )
                            nc.tensor.matmul(
                                rps[:, hi:IC], ones_sb[:], pt[:, 0, hi:IC],
                                start=first, stop=False,
                                skip_group_check=True,
                            )
                        nc.tensor.matmul(
                            cps[:, lo:hi], vnatc[c][:, off, :], pt[:, 0, lo:hi],
                            start=first, stop=is_last, skip_group_check=True,
                        )
                        nc.tensor.matmul(
                            rps[:, lo:hi], ones_sb[:], pt[:, 0, lo:hi],
                            start=first, stop=is_last, skip_group_check=True,
                        )

                    def finalize():
                        rsf = rs_pool.tile([1, IC], F32, name="rsf", tag="rsf")
                        nc.vector.reciprocal(rsf[:], rps[:])
                        bc = bc_pool.tile([128, IC], F32, name="bc", tag="bc")
                        nc.gpsimd.partition_broadcast(bc[:], rsf[:])
                        nc.vector.tensor_mul(ctxc[c][:, h, :], cps[:], bc[:])

                    pairs = [(scores_pair, tail_pair, g) for g in range(2 * c)]
                    diags = [(scores_diag, tail_diag, off) for off in range(4)]
                    if len(pairs) >= 6:
                        # long chunks: spread the cheap diag units among the
                        # later pairs so ACT's exp queue never bunches
                        units = pairs[:2]
                        rest = pairs[2:]
                        for i in range(4):
                            if i < len(rest):
                                units.append(rest[i])
                            units.append(diags[i])
                        units.extend(rest[4:])
                    else:
                        units = pairs + diags
                    return units, finalize

            # Global software pipeline, lookahead 2 across ALL chunks: unit
            # i+2's score matmuls are emitted before unit i's ctx/rowsum so
            # the PE has ~2 units of exp-independent work while an exp is in
            # flight — including at chunk boundaries. (Score-psum buffer
            # reuse only needs exp(i) done before unit i+2's scores run,
            # which the same spacing guarantees.)
            from collections import deque

            pend = deque()
            for c in range(NIC):
                for h in range(HPC):
                    units, fin = chunk_units(c, h)
                    for i, (sc_fn, tl_fn, arg) in enumerate(units):
                        pt = sc_fn(arg)
                        if len(pend) >= 2:
                            p = pend.popleft()
                            p[1](p[2], p[0], is_last=p[3] is not None)
                            if p[3] is not None:
                                p[3]()
                        pend.append((pt, tl_fn, arg,
                                     fin if i == len(units) - 1 else None))
            while pend:
                p = pend.popleft()
                p[1](p[2], p[0], is_last=p[3] is not None)
                if p[3] is not None:
                    p[3]()

        if taps is not None:
            for c in range(NIC):
                nc.sync.dma_start(
                    taps["dbg_q"][:, :, c * IC : (c + 1) * IC], qTc[c][:]
                )
                nc.sync.dma_start(
                    taps["dbg_k"][:, c * IC : (c + 1) * IC], kTc[c][:]
                )
                nc.sync.dma_start(
                    taps["dbg_v"][:, c * 4 : (c + 1) * 4, :], vnatc[c][:]
                )
                nc.sync.dma_start(
                    taps["dbg_ctx"][:, :, c * IC : (c + 1) * IC], ctxc[c][:]
                )

        # ---------------- phase 3: output projection ----------------
        with ExitStack() as p3:
            e = p3.enter_context
            p3_psum = e(tc.tile_pool(name="p3ps", bufs=4, space="PSUM"))
            ev_pool = e(tc.tile_pool(name="ev", bufs=8))
            for c in range(NIC):
                s0 = c * IC
                for ob in range(D // 128):
                    ps = p3_psum.tile([128, IC], F32, name="p3a", tag="p3a")
                    for h in range(HPC):
                        nc.tensor.matmul(
                            ps[:],
                            woT_sb[:, h, ob * 128 : (ob + 1) * 128],
                            ctxc[c][:, h, :],
                            start=(h == 0),
                            stop=(h == HPC - 1),
                        )
                    ev = ev_pool.tile([128, IC], BF, name="ev", tag="ev")
                    nc.scalar.activation(
                        ev[:], ps[:], mybir.ActivationFunctionType.Copy
                    )
                    nc.sync.dma_start(
                        outT[ob * 128 : (ob + 1) * 128, s0 : s0 + IC], ev[:]
                    )


def _rope(nc, tc, rtmp_pool, ps, out_ap, cos_ap, sins_ap):
    """out = psum*cos + rot(psum)*sin_signed, written as bf16.

    sins_ap holds [-sin_lo; sin_hi]; rot(q) = [q_hi; q_lo] (64-row swap).
    """
    n = ps.shape[-1]
    tmp = rtmp_pool.tile([128, n], F32, name="ropetmp", tag="ropetmp")
    t2 = rtmp_pool.tile([128, n], F32, name="ropet2", tag="ropet2")
    nc.vector.tensor_mul(tmp[:], ps[:], cos_ap)
    nc.vector.tensor_mul(t2[:64, :], ps[64:, :], sins_ap[:64, :])
    nc.vector.tensor_mul(t2[64:, :], ps[:64, :], sins_ap[64:, :])
    nc.vector.tensor_add(out_ap, tmp[:], t2[:])


_NC_CACHE = None


def _get_nc():
    global _NC_CACHE
    if _NC_CACHE is None:
        _NC_CACHE = _build_nc()
    return _NC_CACHE


def prepare_in_maps(x, cos, sin, Wq, Wk, Wv, Wo):
    x = np.asarray(x, np.float32)
    cos = np.asarray(cos, np.float32)
    sin = np.asarray(sin, np.float32)
    Wq = np.asarray(Wq, np.float32)
    Wk = np.asarray(Wk, np.float32)
    Wv = np.asarray(Wv, np.float32)
    Wo = np.asarray(Wo, np.float32)

    import ml_dtypes

    bf = ml_dtypes.bfloat16
    xT_bf = np.ascontiguousarray(x[0].T).astype(bf)
    cosT = np.ascontiguousarray(cos.T).astype(bf)            # [128, S] bf16
    sinT = sin.T
    sinTs = np.ascontiguousarray(
        np.concatenate([-sinT[:64], sinT[64:]], axis=0)
    ).astype(bf)

    # causal triangle for a single [key 128, query 128] diagonal sub-block
    j = np.arange(128)[:, None]
    i = np.arange(128)[None, :]
    masks = (i >= j).astype(np.float32).astype(bf)

    def sbuf_layout(wT, width):
        # [D, width] -> [128, DCH, width] (partition-major SBUF image)
        return np.ascontiguousarray(
            wT.reshape(DCH, 128, width).transpose(1, 0, 2)
        ).astype(bf)

    in_maps = []
    for c in range(NCORES):
        wq_c = Wq[c * QO : (c + 1) * QO]          # [512, 4096]
        wk_c = Wk[c * HD : (c + 1) * HD]          # [128, 4096]
        wv_c = Wv[c * HD : (c + 1) * HD]
        wo_c = Wo[:, c * QO : (c + 1) * QO]       # [4096, 512]
        woT_pre = np.ascontiguousarray(
            wo_c.T.reshape(HPC, 128, D).transpose(1, 0, 2)
        ).astype(bf)                               # [128, HPC, D]
        in_maps.append(
            {
                "xT": xT_bf,
                "cosT": cosT,
                "sinTs": sinTs,
                "wqT": np.ascontiguousarray(
                    sbuf_layout(wq_c.T, QO)
                    .reshape(128, DCH, HPC, HD)
                    .transpose(2, 0, 1, 3)
                ),
                "wkT": sbuf_layout(wk_c.T, HD),
                "wvT": sbuf_layout(wv_c.T, HD),
                "woT": woT_pre,
                "masks": masks,
            }
        )
    return in_maps


def kernel(x, cos, sin, Wq, Wk, Wv, Wo, _trace=False):
    nc = _get_nc()
    in_maps = prepare_in_maps(x, cos, sin, Wq, Wk, Wv, Wo)
    res = run_bass_kernel_spmd(
        nc, in_maps, core_ids=list(range(NCORES)), trace=_trace
    )
    acc = np.zeros((D, S), np.float32)
    for r in res.results:
        acc += r["outT"].astype(np.float32)
    out = np.ascontiguousarray(acc.T)[None]      # [1, S, D] fp32
    if _trace:
        return out, res
    return out



# revision 68
# speedup vs baseline: 2.9580x; 1.5347x over previous
"""GQA (32 Q heads / 8 KV heads, S=2048, D=4096, hd=128) on 8 TRN2 cores.

Tensor-parallel over heads: core c owns Q heads [4c, 4c+4) and KV head c.
Per-core bass kernel computes, fully on-chip (bf16 matmuls, fp32 accum):
  phase 1: qT/kT/vT projections (transposed layout [feat, seq]) + RoPE,
           with DMAs queued in exact consumption order (startup is
           DMA-bandwidth-bound)
  phase 2: causal attention, scoresT[j,i] layout, exp-softmax without
           max-subtraction (logits are small), PE ones-matmul row sums.
           Off-diagonal key blocks processed in pairs sharing one batched
           exp; diagonal blocks stream only causally-valid query columns;
           globally software-pipelined with lookahead 2 so the PE always
           has exp-independent work.
  phase 3: partial output projection -> outT [4096, 2048] (bf16)
Host sums the 8 partial outputs in fp32 and transposes back.

All layout transposes (x.T, W.T) are done on host; weights are
pre-arranged into their exact SBUF images so every weight DMA is one
full-bandwidth contiguous transfer.
"""

import numpy as np

import concourse.bass as bass
import concourse.mybir as mybir
import concourse.tile as tile
from concourse import bacc
from concourse.bass_utils import run_bass_kernel_spmd
from concourse.masks import make_identity

B, S, D = 1, 2048, 4096
N_HEADS, N_KV = 32, 8
HD = 128                      # head dim
GROUP = N_HEADS // N_KV       # 4
NCORES = 8
HPC = N_HEADS // NCORES       # 4 q heads per core
QO = HPC * HD                 # 512 q rows per core
SCALE = 1.0 / np.sqrt(np.float32(HD))

SP = 256                      # phase-1 seq panel width
NSP = S // SP                 # 8 panels
DCH = D // 128                # 32 contraction chunks
IC = 512                      # phase-2 query chunk width
NIC = S // IC                 # 4 query chunks
NJB = S // 128                # 16 key blocks

BF = mybir.dt.bfloat16
F32 = mybir.dt.float32

DEBUG_TAPS = False  # extra dram outputs of qT/kT/vnat/ctx for CoreSim checks


def _build_nc():
    nc = bacc.Bacc("TRN2", target_bir_lowering=False, debug=False)

    xT = nc.dram_tensor("xT", [D, S], BF, kind="ExternalInput")
    cosT = nc.dram_tensor("cosT", [HD, S], BF, kind="ExternalInput")
    sinTs = nc.dram_tensor("sinTs", [HD, S], BF, kind="ExternalInput")
    # weights pre-arranged host-side into their SBUF layouts so each is one
    # big-element (full-bandwidth) DMA
    wqT = nc.dram_tensor("wqT", [HPC, 128, DCH, HD], BF, kind="ExternalInput")
    wkT = nc.dram_tensor("wkT", [128, DCH, HD], BF, kind="ExternalInput")
    wvT = nc.dram_tensor("wvT", [128, DCH, HD], BF, kind="ExternalInput")
    woT = nc.dram_tensor("woT", [128, HPC, D], BF, kind="ExternalInput")
    masks = nc.dram_tensor("masks", [128, 128], BF, kind="ExternalInput")
    outT = nc.dram_tensor("outT", [D, S], BF, kind="ExternalOutput")
    taps = None
    if DEBUG_TAPS:
        taps = {
            "dbg_q": nc.dram_tensor("dbg_q", [128, HPC, S], BF, kind="ExternalOutput"),
            "dbg_k": nc.dram_tensor("dbg_k", [128, S], BF, kind="ExternalOutput"),
            "dbg_v": nc.dram_tensor("dbg_v", [128, NJB, HD], BF, kind="ExternalOutput"),
            "dbg_ctx": nc.dram_tensor("dbg_ctx", [128, HPC, S], BF, kind="ExternalOutput"),
        }

    with tile.TileContext(nc) as tc:
        _emit(nc, tc, xT, cosT, sinTs, wqT, wkT, wvT, woT, masks, outT, taps)
    nc.compile()
    return nc


def _emit(nc, tc, xT, cosT, sinTs, wqT, wkT, wvT, woT, masks, outT, taps=None):
    from contextlib import ExitStack

    with ExitStack() as outer:
        ep = outer.enter_context  # persistent pools

        pers = ep(tc.tile_pool(name="pers", bufs=1))
        # per-chunk tiles so cross-phase deps resolve at chunk granularity
        # (the tile framework tracks whole tiles, not subranges)
        qTc = [pers.tile([128, HPC, IC], BF, name=f"qT{c}") for c in range(NIC)]
        kTc = [pers.tile([128, IC], BF, name=f"kT{c}") for c in range(NIC)]
        vnatc = [
            pers.tile([128, 4, HD], BF, name=f"vnat{c}") for c in range(NIC)
        ]
        ctxc = [
            pers.tile([128, HPC, IC], BF, name=f"ctx{c}") for c in range(NIC)
        ]
        mask_sb = pers.tile([128, 128], BF, name="mask_sb")
        ones_sb = pers.tile([128, 1], BF, name="ones_sb")
        ident = pers.tile([128, 128], BF, name="ident")
        woT_sb = pers.tile([128, HPC, D], BF, name="woT_sb")  # [m, head, o]
        warm_sb = pers.tile([1, 1], F32, name="warm_sb")

        nc.gpsimd.memset(ones_sb[:], 1.0)
        make_identity(nc, ident[:])

        # ---------------- phase 1: projections + RoPE ----------------
        with ExitStack() as p1:
            e = p1.enter_context
            # wq split per head, wk split in d-halves: finer tiles let the
            # first matmuls start as soon as their slice of weights lands
            # (tile deps are whole-tile)
            wq_pool = e(tc.tile_pool(name="wq", bufs=1))
            wq_sbs = [
                wq_pool.tile([128, DCH, HD], BF, name=f"wq_sb{h}")
                for h in range(HPC)
            ]
            wk_pool = e(tc.tile_pool(name="wk", bufs=1))
            wk_sbs = [
                wk_pool.tile([128, DCH // 2, HD], BF, name=f"wk_sb{u}")
                for u in range(2)
            ]
            wv_sb = e(tc.tile_pool(name="wv", bufs=1)).tile(
                [128, DCH, HD], BF, name="wv_sb"
            )
            cs_pool = e(tc.tile_pool(name="cs", bufs=1))
            cosT_sb = cs_pool.tile([128, S], BF, name="cosT_sb")
            sinTs_sb = cs_pool.tile([128, S], BF, name="sinTs_sb")



            xp_pool = e(tc.tile_pool(name="xp", bufs=2))
            p1_psum = e(tc.tile_pool(name="p1ps", bufs=6, space="PSUM"))
            tr_psum = e(tc.tile_pool(name="trps", bufs=2, space="PSUM"))
            rtmp_pool = e(tc.tile_pool(name="rtmp", bufs=2))
            vt_pool = e(tc.tile_pool(name="vt", bufs=2))

            def load_xpan(sp, halves=False):
                src = xT[:, sp * SP : (sp + 1) * SP].rearrange(
                    "(c p) s -> p c s", p=128
                )
                if not halves:
                    t = xp_pool.tile([128, DCH, SP], BF, name="xpan")
                    nc.sync.dma_start(t[:], src)
                    return [(t, 0)]
                # panel 0: two half tiles so the first K matmuls start after
                # only half the panel has landed
                out = []
                for u in range(2):
                    t = xp_pool.tile([128, DCH // 2, SP], BF, name="xpanh")
                    nc.sync.dma_start(
                        t[:], src[:, u * (DCH // 2) : (u + 1) * (DCH // 2), :]
                    )
                    out.append((t, u * (DCH // 2)))
                    if u == 0:
                        nc.sync.dma_start(wk_sbs[1][:], wkT[:, DCH // 2 :, :])
                return out

            def xsel(xpan, d):
                # xpan is a list of (tile, d_base); find the piece holding d
                for t, base in reversed(xpan):
                    if d >= base:
                        return t[:, d - base, :]
                raise AssertionError

            # DMA order = need order: the panel-0 K matmuls need only the
            # first wk/xpan0 halves; V/Q weights and rope tables stream in
            # behind them; wo (phase 3) last.
            nc.sync.dma_start(wk_sbs[0][:], wkT[:, : DCH // 2, :])
            xpan_next = load_xpan(0, halves=True)  # also queues wk_sbs[1]
            nc.sync.dma_start(wv_sb[:], wvT[:])
            nc.sync.dma_start(wq_sbs[0][:], wqT[0])
            nc.sync.dma_start(wq_sbs[1][:], wqT[1])
            nc.sync.dma_start(cosT_sb[:], cosT[:])
            nc.sync.dma_start(sinTs_sb[:], sinTs[:])
            nc.sync.dma_start(wq_sbs[2][:], wqT[2])
            nc.sync.dma_start(wq_sbs[3][:], wqT[3])
            xpan_1 = load_xpan(1)
            nc.sync.dma_start(mask_sb[:], masks[:])

            # Warm-up ops: absorb first-touch DMA waits on DVE/ACT so hot
            # TensorTensor/Activation instructions carry at most one wait
            # (the DVE TT struct rejects >1 sync wait in walrus codegen).
            # Self-writes create RAW deps that force these to run first.
            nc.vector.tensor_copy(cosT_sb[:1, :1], cosT_sb[:1, :1])
            nc.vector.tensor_copy(sinTs_sb[:1, :1], sinTs_sb[:1, :1])
            nc.vector.tensor_copy(mask_sb[:1, :1], mask_sb[:1, :1])
            nc.scalar.activation(
                warm_sb[:], cosT_sb[:1, :1], mybir.ActivationFunctionType.Copy
            )
            nc.scalar.activation(
                warm_sb[:], warm_sb[:], mybir.ActivationFunctionType.Exp
            )

            for sp in range(NSP):
                s0 = sp * SP
                xpan = xpan_next
                if sp == 0:
                    xpan_next = xpan_1  # already queued in the preamble
                elif sp + 1 < NSP:
                    xpan_next = load_xpan(sp + 1)
                if sp == 1:
                    # phase-3 weights, queued once the startup-critical
                    # DMAs are all in flight
                    nc.sync.dma_start(woT_sb[:], woT[:])
                ch, co = sp // 2, (sp % 2) * SP  # chunk tile + col offset

                def k_block(xpan=xpan, s0=s0, ch=ch, co=co):
                    ps = p1_psum.tile([128, SP], F32, name="p1acc", tag="p1acc")
                    for d in range(DCH):
                        nc.tensor.matmul(
                            ps[:], wk_sbs[d // (DCH // 2)][:, d % (DCH // 2), :],
                            xsel(xpan, d),
                            start=(d == 0), stop=(d == DCH - 1),
                        )
                    _rope(nc, tc, rtmp_pool, ps, kTc[ch][:, co : co + SP],
                          cosT_sb[:, s0 : s0 + SP], sinTs_sb[:, s0 : s0 + SP])

                def v_block(xpan=xpan, s0=s0):
                    # evac then PE-transpose to natural [s, hd]
                    ps = p1_psum.tile([128, SP], F32, name="p1acc", tag="p1acc")
                    for d in range(DCH):
                        nc.tensor.matmul(
                            ps[:], wv_sb[:, d, :], xsel(xpan, d),
                            start=(d == 0), stop=(d == DCH - 1),
                        )
                    vt = vt_pool.tile([128, SP], BF, name="vt")
                    nc.scalar.activation(
                        vt[:], ps[:], mybir.ActivationFunctionType.Copy
                    )
                    for b in range(SP // 128):
                        jb = (s0 + b * 128) // 128
                        tp = tr_psum.tile([128, 128], BF, name="trp", tag="trp")
                        nc.tensor.transpose(
                            tp[:], vt[:, b * 128 : (b + 1) * 128], ident[:]
                        )
                        nc.scalar.activation(
                            vnatc[jb // 4][:, jb % 4, :],
                            tp[:],
                            mybir.ActivationFunctionType.Copy,
                        )

                def q_block(h, xpan=xpan, s0=s0, ch=ch, co=co):
                    ps = p1_psum.tile([128, SP], F32, name="p1acc", tag="p1acc")
                    for d in range(DCH):
                        nc.tensor.matmul(
                            ps[:],
                            wq_sbs[h][:, d, :],
                            xsel(xpan, d),
                            start=(d == 0),
                            stop=(d == DCH - 1),
                        )
                    _rope(nc, tc, rtmp_pool, ps, qTc[ch][:, h, co : co + SP],
                          cosT_sb[:, s0 : s0 + SP], sinTs_sb[:, s0 : s0 + SP])

                if sp < NSP - 1:
                    k_block()
                    v_block()
                    for h in range(HPC):
                        q_block(h)
                else:
                    # last panel: Q (long DVE rope chains) first so the final
                    # p1-psum consumers finish close to the last PE matmul —
                    # phase 2's psum pool waits on this pool's release
                    for h in range(HPC):
                        q_block(h)
                    k_block()
                    v_block()

        # ---------------- phase 2: attention ----------------
        # Off-diagonal key blocks are processed in PAIRS sharing one batched
        # exp (halves ACT instruction overhead). Diagonal blocks stream only
        # the causally-valid query columns [off*128:IC] through scores / exp /
        # ctx / rowsum; the remaining triangle is zeroed by a [128,128] mask.
        with ExitStack() as p2:
            e = p2.enter_context
            sc_psum = e(tc.tile_pool(name="scps", bufs=2, space="PSUM"))
            cx_psum = e(tc.tile_pool(name="cxps", bufs=2, space="PSUM"))
            rs_psum = e(tc.tile_pool(name="rsps", bufs=2, space="PSUM"))
            pt_pool = e(tc.tile_pool(name="pt", bufs=6))
            rs_pool = e(tc.tile_pool(name="rs", bufs=2))
            bc_pool = e(tc.tile_pool(name="bc", bufs=2))
            EXP = mybir.ActivationFunctionType.Exp

            def chunk_units(c, h):
                    q_rhs = qTc[c][:, h, :]
                    cps = cx_psum.tile([128, IC], F32, name="cxa", tag="cxa")
                    rps = rs_psum.tile([1, IC], F32, name="rsa", tag="rsa")

                    def scores_pair(g):
                        sps = sc_psum.tile(
                            [128, 2, IC], F32, name="scp", tag="scp"
                        )
                        for u in range(2):
                            jb = 2 * g + u
                            nc.tensor.matmul(
                                sps[:, u, :],
                                kTc[jb // 4][:, (jb % 4) * 128 : (jb % 4 + 1) * 128],
                                q_rhs,
                                start=True,
                                stop=True,
                            )
                        pt = pt_pool.tile([128, 2, IC], BF, name="pt", tag="pt")
                        nc.scalar.activation(pt[:], sps[:], EXP, scale=float(SCALE))
                        return pt

                    def tail_pair(g, pt, is_last=False):
                        for u in range(2):
                            jb = 2 * g + u
                            stop = is_last and u == 1
                            nc.tensor.matmul(
                                cps[:], vnatc[jb // 4][:, jb % 4, :], pt[:, u, :],
                                start=(jb == 0), stop=stop,
                                skip_group_check=True,
                            )
                            nc.tensor.matmul(
                                rps[:], ones_sb[:], pt[:, u, :],
                                start=(jb == 0), stop=stop,
                                skip_group_check=True,
                            )

                    def scores_diag(off):
                        lo = off * 128
                        sps = sc_psum.tile(
                            [128, 2, IC], F32, name="scp", tag="scp"
                        )
                        nc.tensor.matmul(
                            sps[:, 0, lo:IC],
                            kTc[c][:, lo : lo + 128],
                            q_rhs[:, lo:IC],
                            start=True,
                            stop=True,
                        )
                        pt = pt_pool.tile([128, 2, IC], BF, name="pt", tag="pt")
                        nc.scalar.activation(
                            pt[:, 0, lo:IC], sps[:, 0, lo:IC], EXP,
                            scale=float(SCALE),
                        )
                        nc.vector.tensor_mul(
                            pt[:, 0, lo : lo + 128],
                            pt[:, 0, lo : lo + 128],
                            mask_sb[:],
                        )
                        return pt

                    def tail_diag(off, pt, is_last=False):
                        lo = off * 128
                        first = c == 0 and off == 0
                        hi = lo + 128
                        if first:
                            # single full-width start=True write: a second
                            # start=True into the same psum bank would zero
                            # the previously written region
                            nc.tensor.matmul(
                                cps[:, lo:IC], vnatc[c][:, off, :],
                                pt[:, 0, lo:IC],
                                start=True, stop=is_last,
                                skip_group_check=True,
                            )
                            nc.tensor.matmul(
                                rps[:, lo:IC], ones_sb[:], pt[:, 0, lo:IC],
                                start=True, stop=is_last,
                                skip_group_check=True,
                            )
                            return
                        if hi < IC:
                            # mask-free columns: depend on exp only, keeping
                            # PE fed while the triangle mask runs on DVE
                            nc.tensor.matmul(
                                cps[:, hi:IC], vnatc[c][:, off, :],
                                pt[:, 0, hi:IC],
                                start=first, stop=False,
                                skip_group_check=True,
                            )
                            nc.tensor.matmul(
                                rps[:, hi:IC], ones_sb[:], pt[:, 0, hi:IC],
                                start=first, stop=False,
                                skip_group_check=True,
                            )
                        nc.tensor.matmul(
                            cps[:, lo:hi], vnatc[c][:, off, :], pt[:, 0, lo:hi],
                            start=first, stop=is_last, skip_group_check=True,
                        )
                        nc.tensor.matmul(
                            rps[:, lo:hi], ones_sb[:], pt[:, 0, lo:hi],
                            start=first, stop=is_last, skip_group_check=True,
                        )

                    def finalize():
                        rsf = rs_pool.tile([1, IC], F32, name="rsf", tag="rsf")
                        nc.vector.reciprocal(rsf[:], rps[:])
                        bc = bc_pool.tile([128, IC], F32, name="bc", tag="bc")
                        nc.gpsimd.partition_broadcast(bc[:], rsf[:])
                        nc.vector.tensor_mul(ctxc[c][:, h, :], cps[:], bc[:])

                    pairs = [(scores_pair, tail_pair, g) for g in range(2 * c)]
                    diags = [(scores_diag, tail_diag, off) for off in range(4)]
                    if len(pairs) >= 6:
                        # long chunks: spread the cheap diag units among the
                        # later pairs so ACT's exp queue never bunches
                        units = pairs[:2]
                        rest = pairs[2:]
                        for i in range(4):
                            if i < len(rest):
                                units.append(rest[i])
                            units.append(diags[i])
                        units.extend(rest[4:])
                    else:
                        units = pairs + diags
                    return units, finalize

            # Global software pipeline, lookahead 2 across ALL chunks: unit
            # i+2's score matmuls are emitted before unit i's ctx/rowsum so
            # the PE has ~2 units of exp-independent work while an exp is in
            # flight — including at chunk boundaries. (Score-psum buffer
            # reuse only needs exp(i) done before unit i+2's scores run,
            # which the same spacing guarantees.)
            from collections import deque

            pend = deque()
            for c in range(NIC):
                for h in range(HPC):
                    units, fin = chunk_units(c, h)
                    for i, (sc_fn, tl_fn, arg) in enumerate(units):
                        pt = sc_fn(arg)
                        if len(pend) >= 2:
                            p = pend.popleft()
                            p[1](p[2], p[0], is_last=p[3] is not None)
                            if p[3] is not None:
                                p[3]()
                        pend.append((pt, tl_fn, arg,
                                     fin if i == len(units) - 1 else None))
            while pend:
                p = pend.popleft()
                p[1](p[2], p[0], is_last=p[3] is not None)
                if p[3] is not None:
                    p[3]()

        if taps is not None:
            for c in range(NIC):
                nc.sync.dma_start(
                    taps["dbg_q"][:, :, c * IC : (c + 1) * IC], qTc[c][:]
                )
                nc.sync.dma_start(
                    taps["dbg_k"][:, c * IC : (c + 1) * IC], kTc[c][:]
                )
                nc.sync.dma_start(
                    taps["dbg_v"][:, c * 4 : (c + 1) * 4, :], vnatc[c][:]
                )
                nc.sync.dma_start(
                    taps["dbg_ctx"][:, :, c * IC : (c + 1) * IC], ctxc[c][:]
                )

        # ---------------- phase 3: output projection ----------------
        with ExitStack() as p3:
            e = p3.enter_context
            p3_psum = e(tc.tile_pool(name="p3ps", bufs=4, space="PSUM"))
            ev_pool = e(tc.tile_pool(name="ev", bufs=8))
            for c in range(NIC):
                s0 = c * IC
                for ob in range(D // 128):
                    ps = p3_psum.tile([128, IC], F32, name="p3a", tag="p3a")
                    for h in range(HPC):
                        nc.tensor.matmul(
                            ps[:],
                            woT_sb[:, h, ob * 128 : (ob + 1) * 128],
                            ctxc[c][:, h, :],
                            start=(h == 0),
                            stop=(h == HPC - 1),
                        )
                    ev = ev_pool.tile([128, IC], BF, name="ev", tag="ev")
                    nc.scalar.activation(
                        ev[:], ps[:], mybir.ActivationFunctionType.Copy
                    )
                    nc.sync.dma_start(
                        outT[ob * 128 : (ob + 1) * 128, s0 : s0 + IC], ev[:]
                    )


def _rope(nc, tc, rtmp_pool, ps, out_ap, cos_ap, sins_ap):
    """out = psum*cos + rot(psum)*sin_signed, written as bf16.

    sins_ap holds [-sin_lo; sin_hi]; rot(q) = [q_hi; q_lo] (64-row swap).
    """
    n = ps.shape[-1]
    tmp = rtmp_pool.tile([128, n], F32, name="ropetmp", tag="ropetmp")
    t2 = rtmp_pool.tile([128, n], F32, name="ropet2", tag="ropet2")
    nc.vector.tensor_mul(tmp[:], ps[:], cos_ap)
    nc.vector.tensor_mul(t2[:64, :], ps[64:, :], sins_ap[:64, :])
    nc.vector.tensor_mul(t2[64:, :], ps[:64, :], sins_ap[64:, :])
    nc.vector.tensor_add(out_ap, tmp[:], t2[:])


_NC_CACHE = None


def _get_nc():
    global _NC_CACHE
    if _NC_CACHE is None:
        _NC_CACHE = _build_nc()
    return _NC_CACHE


def prepare_in_maps(x, cos, sin, Wq, Wk, Wv, Wo):
    x = np.asarray(x, np.float32)
    cos = np.asarray(cos, np.float32)
    sin = np.asarray(sin, np.float32)
    Wq = np.asarray(Wq, np.float32)
    Wk = np.asarray(Wk, np.float32)
    Wv = np.asarray(Wv, np.float32)
    Wo = np.asarray(Wo, np.float32)

    import ml_dtypes

    bf = ml_dtypes.bfloat16
    xT_bf = np.ascontiguousarray(x[0].T).astype(bf)
    cosT = np.ascontiguousarray(cos.T).astype(bf)            # [128, S] bf16
    sinT = sin.T
    sinTs = np.ascontiguousarray(
        np.concatenate([-sinT[:64], sinT[64:]], axis=0)
    ).astype(bf)

    # causal triangle for a single [key 128, query 128] diagonal sub-block
    j = np.arange(128)[:, None]
    i = np.arange(128)[None, :]
    masks = (i >= j).astype(np.float32).astype(bf)

    def sbuf_layout(wT, width):
        # [D, width] -> [128, DCH, width] (partition-major SBUF image)
        return np.ascontiguousarray(
            wT.reshape(DCH, 128, width).transpose(1, 0, 2)
        ).astype(bf)

    in_maps = []
    for c in range(NCORES):
        wq_c = Wq[c * QO : (c + 1) * QO]          # [512, 4096]
        wk_c = Wk[c * HD : (c + 1) * HD]          # [128, 4096]
        wv_c = Wv[c * HD : (c + 1) * HD]
        wo_c = Wo[:, c * QO : (c + 1) * QO]       # [4096, 512]
        woT_pre = np.ascontiguousarray(
            wo_c.T.reshape(HPC, 128, D).transpose(1, 0, 2)
        ).astype(bf)                               # [128, HPC, D]
        in_maps.append(
            {
                "xT": xT_bf,
                "cosT": cosT,
                "sinTs": sinTs,
                "wqT": np.ascontiguousarray(
                    sbuf_layout(wq_c.T, QO)
                    .reshape(128, DCH, HPC, HD)
                    .transpose(2, 0, 1, 3)
                ),
                "wkT": sbuf_layout(wk_c.T, HD),
                "wvT": sbuf_layout(wv_c.T, HD),
                "woT": woT_pre,
                "masks": masks,
            }
        )
    return in_maps


def kernel(x, cos, sin, Wq, Wk, Wv, Wo, _trace=False):
    nc = _get_nc()
    in_maps = prepare_in_maps(x, cos, sin, Wq, Wk, Wv, Wo)
    res = run_bass_kernel_spmd(
        nc, in_maps, core_ids=list(range(NCORES)), trace=_trace
    )
    acc = np.zeros((D, S), np.float32)
    for r in res.results:
        acc += r["outT"].astype(np.float32)
    out = np.ascontiguousarray(acc.T)[None]      # [1, S, D] fp32
    if _trace:
        return out, res
    return out

